# revision 2
# baseline (speedup 1.0000x reference)
"""J-regularized cross-entropy loss on 8 Trainium2 cores.

Math: for pred (B,C,H,W) f32, target (B,H,W) int, C=8:
  S[b,k,ci]   = sum_p pred[b,ci,p] * (target[b,p]==k)   (8x8 per batch)
  n[b,k]      = |{p: target[b,p]==k}|
  lse[b,p]    = log sum_c exp(pred[b,c,p])
  M[b,ci,ck]  = S[b,ck,ci]/n[b,ck];  jl = mean_b -sum_{ci!=ck} log(.5+.5*(diag-M))
  ce          = (sum lse - sum_b sum_k S[b,k,k]) / (B*N)
  out         = jl + ce

Device (per core, 2 batches): S via PE matmuls (one-hot weights x pred,
contracting 128 pixels/matmul, PSUM-accumulated), lse via ACT exp + DVE
add-tree + ACT ln with fused accum_out reduction. Inputs pre-converted to
bf16 on host (final scalar error ~1e-5 relative). Host finishes the tiny
(B,8,8) math in f64.

Device pred layout is pixel-major (p, t, c) so each matmul's moving
operand is a single contiguous 128-element free dim (BIR requires 1 free
dim on rhs). One-hot weights are built dg-contiguous: (p, d, k, g) so
lhsT per dg is also one contiguous 128-element slice.
"""

import numpy as np
import ml_dtypes

import concourse.bacc as bacc
import concourse.mybir as mybir
import concourse.tile as tile
from concourse import bass_utils

N_CORES = 8
B, C, H, W = 16, 8, 512, 512
N = H * W                 # 262144 pixels per batch
P = 128                   # SBUF partitions
COLS = N // P             # 2048 pixel-columns per batch
F = 1024                  # pixel-columns per chunk
CH = COLS // F            # chunks per batch
BPC = B // N_CORES        # batches per core
G = 16                    # pixel-columns per matmul group (16*8=128)
NDG = F // G              # matmuls per chunk

TRACE = False             # set True from test.py to neuron-profile
LAST_EXEC_NS = None
LAST_TRACE = None

_BF16 = mybir.dt.bfloat16
_F32 = mybir.dt.float32

_nc_cache = None


def _build_nc():
    nc = bacc.Bacc("TRN2", target_bir_lowering=False, debug=False,
                   num_devices=N_CORES)
    pred_d = nc.dram_tensor("pred", (BPC, CH, P, F * C), _BF16,
                            kind="ExternalInput")
    tgt_d = nc.dram_tensor("target", (BPC, P, COLS), _BF16,
                           kind="ExternalInput")
    smat_d = nc.dram_tensor("smat", (BPC, P, C * G), _F32,
                            kind="ExternalOutput")
    lse_d = nc.dram_tensor("lse", (P, BPC * CH), _F32,
                           kind="ExternalOutput")

    with tile.TileContext(nc) as tc:
        with (
            tc.tile_pool(name="pred", bufs=3) as pred_pool,
            tc.tile_pool(name="oh", bufs=2) as oh_pool,
            tc.tile_pool(name="exp", bufs=2) as exp_pool,
            tc.tile_pool(name="small", bufs=2) as small_pool,
            tc.tile_pool(name="acc", bufs=1) as acc_pool,
            tc.tile_pool(name="psum", bufs=2, space="PSUM") as psum_pool,
        ):
            lse_acc = acc_pool.tile([P, BPC * CH], _F32)
            for b in range(BPC):
                tgt_t = small_pool.tile([P, COLS], _BF16, tag="tgt")
                nc.sync.dma_start(tgt_t[:, :], tgt_d[b])
                psum_t = psum_pool.tile([P, C * G], _F32)
                for ch in range(CH):
                    pred_t = pred_pool.tile([P, F * C], _BF16)
                    nc.sync.dma_start(pred_t[:, :], pred_d[b, ch])

                    # one-hot weights: oh[p, d*128 + k*16 + g] = (tgt==k)
                    oh_t = oh_pool.tile([P, NDG * C * G], _BF16)
                    oh4 = oh_t[:, :].rearrange("p (d k g) -> p d k g",
                                               k=C, g=G)
                    tgt3 = tgt_t[:, ch * F:(ch + 1) * F].rearrange(
                        "p (d g) -> p d g", g=G)
                    for k in range(C):
                        nc.vector.tensor_scalar(
                            oh4[:, :, k, :], tgt3,
                            float(k), None, mybir.AluOpType.is_equal,
                        )

                    # S: psum[k*16+g, g'*8+ci] += oh_dg^T @ pred_dg
                    for d in range(NDG):
                        nc.tensor.matmul(
                            psum_t[:, :],
                            oh_t[:, d * 128:(d + 1) * 128],
                            pred_t[:, d * 128:(d + 1) * 128],
                            start=(ch == 0 and d == 0),
                            stop=(ch == CH - 1 and d == NDG - 1),
                        )

                    # lse: exp (transposed read -> class-major out),
                    # pairwise add tree (bf16 2x mode), ln with fused sum
                    exp_t = exp_pool.tile([P, C, F], _BF16)
                    pred_ct = pred_t[:, :].rearrange("p (t c) -> p c t", c=C)
                    nc.scalar.activation(exp_t[:, :, :], pred_ct,
                                         mybir.ActivationFunctionType.Exp)
                    tmp1 = small_pool.tile([P, 4, F], _BF16, tag="tmp1")
                    nc.vector.tensor_add(tmp1[:, :, :], exp_t[:, 0::2, :],
                                         exp_t[:, 1::2, :])
                    tmp2 = small_pool.tile([P, 2, F], _BF16, tag="tmp2")
                    nc.vector.tensor_add(tmp2[:, :, :], tmp1[:, 0::2, :],
                                         tmp1[:, 1::2, :])
                    sume = small_pool.tile([P, F], _BF16, tag="sume")
                    nc.vector.tensor_add(sume[:, :], tmp2[:, 0, :],
                                         tmp2[:, 1, :])
                    lnsc = small_pool.tile([P, F], _BF16, tag="lnsc")
                    nc.scalar.activation(
                        lnsc[:, :], sume[:, :],
                        mybir.ActivationFunctionType.Ln,
                        accum_out=lse_acc[:, b * CH + ch:b * CH + ch + 1],
                    )

                smat_sb = small_pool.tile([P, C * G], _F32, tag="smat")
                nc.vector.tensor_copy(smat_sb[:, :], psum_t[:, :])
                nc.sync.dma_start(smat_d[b], smat_sb[:, :])
            nc.sync.dma_start(lse_d[:, :], lse_acc[:, :])

    nc.compile()
    return nc


def kernel(pred, target):
    global LAST_EXEC_NS, LAST_TRACE, _nc_cache
    pred = np.asarray(pred)
    target = np.asarray(target)

    if _nc_cache is None:
        _nc_cache = _build_nc()
    nc = _nc_cache

    # pixel-major device layout: (b, ch, p, t, c)
    predv = np.asarray(pred, dtype=np.float32).reshape(B, C, P, CH, F)
    tgtf = target.reshape(B, P, COLS)
    in_maps = []
    for core in range(N_CORES):
        bs = slice(core * BPC, (core + 1) * BPC)
        pc = predv[bs].transpose(0, 3, 2, 4, 1)          # (BPC, CH, P, F, C)
        pc = np.ascontiguousarray(pc).astype(ml_dtypes.bfloat16)
        pc = pc.reshape(BPC, CH, P, F * C)
        tcore = tgtf[bs].astype(np.float32).astype(ml_dtypes.bfloat16)
        in_maps.append({"pred": pc, "target": tcore})

    res = bass_utils.run_bass_kernel_spmd(
        nc, in_maps, core_ids=list(range(N_CORES)), trace=TRACE)
    LAST_EXEC_NS = res.exec_time_ns
    LAST_TRACE = (res.instructions_and_trace[1]
                  if res.instructions_and_trace else None)

    # host combine (tiny): S[b,k,ci] = sum_g smat[k*16+g, g*8+ci]
    S = np.zeros((B, C, C), np.float64)
    total_lse = 0.0
    for core in range(N_CORES):
        smat = res.results[core]["smat"].reshape(BPC, C, G, G, C)
        S[core * BPC:(core + 1) * BPC] = np.einsum(
            "bkggc->bkc", smat.astype(np.float64))
        total_lse += res.results[core]["lse"].astype(np.float64).sum()

    n = np.zeros((B, C), np.float64)
    for b in range(B):
        n[b] = np.bincount(target[b].ravel().astype(np.int64), minlength=C)

    M = S.transpose(0, 2, 1) / n[:, None, :]             # M[b,ci,ck]
    diag = np.einsum("bcc->bc", M)
    inner = (diag[:, :, None] - M) * 0.5
    off = 1.0 - np.eye(C)
    jl = (-(np.log(0.5 + inner) * off).sum(axis=(1, 2))).mean()
    ce = (total_lse - np.einsum("bkk->", S)) / (B * N)
    return np.float32(jl + ce)


# revision 4
# speedup vs baseline: 1.6488x; 1.6488x over previous
"""J-regularized cross-entropy loss on 8 Trainium2 cores.

Math: for pred (B,C,H,W) f32, target (B,H,W) int, C=8:
  S[b,k,ci]   = sum_p pred[b,ci,p] * (target[b,p]==k)   (8x8 per batch)
  n[b,k]      = |{p: target[b,p]==k}|
  lse[b,p]    = log sum_c exp(pred[b,c,p])
  M[b,ci,ck]  = S[b,ck,ci]/n[b,ck];  jl = mean_b -sum_{ci!=ck} log(.5+.5*(diag-M))
  ce          = (sum lse - sum_b sum_k S[b,k,k]) / (B*N)
  out         = jl + ce

Device (per core, 2 batches): S via PE matmuls (one-hot weights x pred,
contracting 128 pixels/matmul, PSUM-accumulated), lse via ACT exp + DVE
add-tree + ACT ln with fused accum_out reduction. Inputs pre-converted to
bf16 on host (final scalar error ~1e-5 relative). Host finishes the tiny
(B,8,8) math in f64.

Device pred layout is pixel-major (p, t, c) so each matmul's moving
operand is a single contiguous 128-element free dim (BIR requires 1 free
dim on rhs). One-hot weights are built dg-contiguous: (p, d, k, g) so
lhsT per dg is also one contiguous 128-element slice.
"""

import numpy as np
import ml_dtypes

import concourse.bacc as bacc
import concourse.mybir as mybir
import concourse.tile as tile
from concourse import bass_utils

N_CORES = 8
B, C, H, W = 16, 8, 512, 512
N = H * W                 # 262144 pixels per batch
P = 128                   # SBUF partitions
COLS = N // P             # 2048 pixel-columns per batch
F = 1024                  # pixel-columns per chunk
CH = COLS // F            # chunks per batch
BPC = B // N_CORES        # batches per core
G = 16                    # pixel-columns per matmul group (16*8=128)
NDG = F // G              # matmuls per chunk

TRACE = False             # set True from test.py to neuron-profile
LAST_EXEC_NS = None
LAST_TRACE = None

_BF16 = mybir.dt.bfloat16
_F32 = mybir.dt.float32

_nc_cache = None


def _build_nc():
    nc = bacc.Bacc("TRN2", target_bir_lowering=False, debug=False,
                   num_devices=N_CORES)
    pred_d = nc.dram_tensor("pred", (BPC, CH, P, F * C), _BF16,
                            kind="ExternalInput")
    tgt_d = nc.dram_tensor("target", (BPC, P, COLS), _BF16,
                           kind="ExternalInput")
    smat_d = nc.dram_tensor("smat", (BPC, P, C * G), _F32,
                            kind="ExternalOutput")
    lse_d = nc.dram_tensor("lse", (P, BPC), _F32,
                           kind="ExternalOutput")

    with tile.TileContext(nc) as tc:
        with (
            tc.tile_pool(name="pred", bufs=3) as pred_pool,
            tc.tile_pool(name="oh", bufs=2) as oh_pool,
            tc.tile_pool(name="exp", bufs=2) as exp_pool,
            tc.tile_pool(name="small", bufs=2) as small_pool,
            tc.tile_pool(name="acc", bufs=1) as acc_pool,
            tc.tile_pool(name="psum", bufs=2, space="PSUM") as psum_pool,
        ):
            lse_acc = acc_pool.tile([P, BPC], _F32)
            sume_all = []
            psums = []
            for b in range(BPC):
                tgt_t = small_pool.tile([P, COLS], _BF16, tag="tgt")
                nc.sync.dma_start(tgt_t[:, :], tgt_d[b])
                psum_t = psum_pool.tile([P, C * G], _F32)
                psums.append(psum_t)
                sume_b = acc_pool.tile([P, COLS], _BF16, tag=f"sume{b}")
                sume_all.append(sume_b)
                for ch in range(CH):
                    pred_t = pred_pool.tile([P, F * C], _BF16)
                    nc.sync.dma_start(pred_t[:, :], pred_d[b, ch])

                    # one-hot weights: oh[p, d*128 + k*16 + g] = (tgt==k)
                    oh_t = oh_pool.tile([P, NDG * C * G], _BF16)
                    oh4 = oh_t[:, :].rearrange("p (d k g) -> p d k g",
                                               k=C, g=G)
                    tgt3 = tgt_t[:, ch * F:(ch + 1) * F].rearrange(
                        "p (d g) -> p d g", g=G)
                    for k in range(C):
                        nc.vector.tensor_scalar(
                            oh4[:, :, k, :], tgt3,
                            float(k), None, mybir.AluOpType.is_equal,
                        )

                    # S: psum[k*16+g, g'*8+ci] += oh_dg^T @ pred_dg
                    for d in range(NDG):
                        nc.tensor.matmul(
                            psum_t[:, :],
                            oh_t[:, d * 128:(d + 1) * 128],
                            pred_t[:, d * 128:(d + 1) * 128],
                            start=(ch == 0 and d == 0),
                            stop=(ch == CH - 1 and d == NDG - 1),
                        )

                    # lse: exp contiguous pixel-major; class-sum via a
                    # half-split add tree whose slices stay step-1 so the
                    # bf16 DVE 2x mode applies (L1: +4 offset, L2: +2, L3: +1)
                    exp_t = exp_pool.tile([P, F * C], _BF16)
                    nc.scalar.activation(exp_t[:, :], pred_t[:, :],
                                         mybir.ActivationFunctionType.Exp)
                    e3 = exp_t[:, :].rearrange("p (t c) -> p t c", c=C)
                    tmp1 = small_pool.tile([P, F, 4], _BF16, tag="tmp1")
                    nc.vector.tensor_add(tmp1[:, :, :], e3[:, :, 0:4],
                                         e3[:, :, 4:8])
                    tmp2 = small_pool.tile([P, F, 2], _BF16, tag="tmp2")
                    nc.vector.tensor_add(tmp2[:, :, :], tmp1[:, :, 0:2],
                                         tmp1[:, :, 2:4])
                    nc.vector.tensor_add(
                        sume_b[:, ch * F:(ch + 1) * F],
                        tmp2[:, :, 0], tmp2[:, :, 1])

            # all Ln after all Exp: one ACT table-set switch instead of four
            for b in range(BPC):
                lnsc = small_pool.tile([P, COLS], _BF16, tag="lnsc")
                nc.scalar.activation(
                    lnsc[:, :], sume_all[b][:, :],
                    mybir.ActivationFunctionType.Ln,
                    accum_out=lse_acc[:, b:b + 1],
                )
                smat_sb = small_pool.tile([P, C * G], _F32, tag="smat")
                nc.vector.tensor_copy(smat_sb[:, :], psums[b][:, :])
                nc.sync.dma_start(smat_d[b], smat_sb[:, :])
            nc.sync.dma_start(lse_d[:, :], lse_acc[:, :])

    nc.compile()
    return nc


def kernel(pred, target):
    global LAST_EXEC_NS, LAST_TRACE, _nc_cache
    pred = np.asarray(pred)
    target = np.asarray(target)

    if _nc_cache is None:
        _nc_cache = _build_nc()
    nc = _nc_cache

    # pixel-major device layout: (b, ch, p, t, c)
    predv = np.asarray(pred, dtype=np.float32).reshape(B, C, P, CH, F)
    tgtf = target.reshape(B, P, COLS)
    in_maps = []
    for core in range(N_CORES):
        bs = slice(core * BPC, (core + 1) * BPC)
        pc = predv[bs].transpose(0, 3, 2, 4, 1)          # (BPC, CH, P, F, C)
        pc = np.ascontiguousarray(pc).astype(ml_dtypes.bfloat16)
        pc = pc.reshape(BPC, CH, P, F * C)
        tcore = tgtf[bs].astype(np.float32).astype(ml_dtypes.bfloat16)
        in_maps.append({"pred": pc, "target": tcore})

    res = bass_utils.run_bass_kernel_spmd(
        nc, in_maps, core_ids=list(range(N_CORES)), trace=TRACE)
    LAST_EXEC_NS = res.exec_time_ns
    LAST_TRACE = (res.instructions_and_trace[1]
                  if res.instructions_and_trace else None)

    # host combine (tiny): S[b,k,ci] = sum_g smat[k*16+g, g*8+ci]
    S = np.zeros((B, C, C), np.float64)
    total_lse = 0.0
    for core in range(N_CORES):
        smat = res.results[core]["smat"].reshape(BPC, C, G, G, C)
        S[core * BPC:(core + 1) * BPC] = np.einsum(
            "bkggc->bkc", smat.astype(np.float64))
        total_lse += res.results[core]["lse"].astype(np.float64).sum()

    n = np.zeros((B, C), np.float64)
    for b in range(B):
        n[b] = np.bincount(target[b].ravel().astype(np.int64), minlength=C)

    M = S.transpose(0, 2, 1) / n[:, None, :]             # M[b,ci,ck]
    diag = np.einsum("bcc->bc", M)
    inner = (diag[:, :, None] - M) * 0.5
    off = 1.0 - np.eye(C)
    jl = (-(np.log(0.5 + inner) * off).sum(axis=(1, 2))).mean()
    ce = (total_lse - np.einsum("bkk->", S)) / (B * N)
    return np.float32(jl + ce)


# revision 7
# speedup vs baseline: 1.6997x; 1.0309x over previous
"""J-regularized cross-entropy loss on 8 Trainium2 cores.

Math: for pred (B,C,H,W) f32, target (B,H,W) int, C=8:
  S[b,k,ci]   = sum_p pred[b,ci,p] * (target[b,p]==k)   (8x8 per batch)
  n[b,k]      = |{p: target[b,p]==k}|
  lse[b,p]    = log sum_c exp(pred[b,c,p])
  M[b,ci,ck]  = S[b,ck,ci]/n[b,ck];  jl = mean_b -sum_{ci!=ck} log(.5+.5*(diag-M))
  ce          = (sum lse - sum_b sum_k S[b,k,k]) / (B*N)
  out         = jl + ce

Device (per core, 2 batches): S via PE matmuls (one-hot weights x pred,
contracting 128 pixels/matmul, PSUM-accumulated), lse via ACT exp + DVE
add-tree + ACT ln with fused accum_out reduction. Inputs pre-converted to
bf16 on host (final scalar error ~1e-5 relative). Host finishes the tiny
(B,8,8) math in f64.

Device pred layout is pixel-major (p, t, c) so each matmul's moving
operand is a single contiguous 128-element free dim (BIR requires 1 free
dim on rhs). One-hot weights are built dg-contiguous: (p, d, k, g) so
lhsT per dg is also one contiguous 128-element slice.
"""

import numpy as np
import ml_dtypes

import concourse.bacc as bacc
import concourse.mybir as mybir
import concourse.tile as tile
from concourse import bass_utils

N_CORES = 8
B, C, H, W = 16, 8, 512, 512
N = H * W                 # 262144 pixels per batch
P = 128                   # SBUF partitions
COLS = N // P             # 2048 pixel-columns per batch
F = 1024                  # pixel-columns per chunk
CH = COLS // F            # chunks per batch
BPC = B // N_CORES        # batches per core
G = 16                    # pixel-columns per matmul group (16*8=128)
NDG = F // G              # matmuls per chunk

TRACE = False             # set True from test.py to neuron-profile
LAST_EXEC_NS = None
LAST_TRACE = None

_BF16 = mybir.dt.bfloat16
_F32 = mybir.dt.float32

_nc_cache = None


def _build_nc():
    nc = bacc.Bacc("TRN2", target_bir_lowering=False, debug=False,
                   num_devices=N_CORES)
    pred_d = nc.dram_tensor("pred", (BPC, CH, P, F * C), _BF16,
                            kind="ExternalInput")
    tgt_d = nc.dram_tensor("target", (BPC, P, COLS), _BF16,
                           kind="ExternalInput")
    smat_d = nc.dram_tensor("smat", (BPC, P, C * G), _F32,
                            kind="ExternalOutput")
    lse_d = nc.dram_tensor("lse", (P, BPC), _F32,
                           kind="ExternalOutput")

    with tile.TileContext(nc) as tc:
        with (
            tc.tile_pool(name="pred", bufs=3) as pred_pool,
            tc.tile_pool(name="oh", bufs=2) as oh_pool,
            tc.tile_pool(name="exp", bufs=2) as exp_pool,
            tc.tile_pool(name="small", bufs=2) as small_pool,
            tc.tile_pool(name="acc", bufs=1) as acc_pool,
            tc.tile_pool(name="psum", bufs=2, space="PSUM") as psum_pool,
        ):
            lse_acc = acc_pool.tile([P, BPC], _F32)
            sume_all = []
            psums = []
            for b in range(BPC):
                tgt_t = small_pool.tile([P, COLS], _BF16, tag="tgt")
                nc.sync.dma_start(tgt_t[:, :], tgt_d[b])
                psum_t = psum_pool.tile([P, C * G], _F32)
                psums.append(psum_t)
                sume_b = acc_pool.tile([P, COLS], _BF16, tag=f"sume{b}")
                sume_all.append(sume_b)
                for ch in range(CH):
                    pred_t = pred_pool.tile([P, F * C], _BF16)
                    HB = F * C // 2
                    nc.sync.dma_start(pred_t[:, :HB], pred_d[b, ch, :, :HB])
                    nc.sync.dma_start(pred_t[:, HB:], pred_d[b, ch, :, HB:])

                    # one-hot weights: oh[p, d*128 + k*16 + g] = (tgt==k)
                    oh_t = oh_pool.tile([P, NDG * C * G], _BF16)
                    oh4 = oh_t[:, :].rearrange("p (d k g) -> p d k g",
                                               k=C, g=G)
                    tgt3 = tgt_t[:, ch * F:(ch + 1) * F].rearrange(
                        "p (d g) -> p d g", g=G)
                    for k in range(C):
                        nc.vector.tensor_scalar(
                            oh4[:, :, k, :], tgt3,
                            float(k), None, mybir.AluOpType.is_equal,
                        )

                    # S: psum[k*16+g, g'*8+ci] += oh_dg^T @ pred_dg
                    for d in range(NDG):
                        nc.tensor.matmul(
                            psum_t[:, :],
                            oh_t[:, d * 128:(d + 1) * 128],
                            pred_t[:, d * 128:(d + 1) * 128],
                            start=(ch == 0 and d == 0),
                            stop=(ch == CH - 1 and d == NDG - 1),
                        )

                    # lse: exp contiguous pixel-major; class-sum via a
                    # half-split add tree whose slices stay step-1 so the
                    # bf16 DVE 2x mode applies (L1: +4 offset, L2: +2, L3: +1)
                    exp_t = exp_pool.tile([P, F * C], _BF16)
                    nc.scalar.activation(exp_t[:, :HB], pred_t[:, :HB],
                                         mybir.ActivationFunctionType.Exp)
                    nc.scalar.activation(exp_t[:, HB:], pred_t[:, HB:],
                                         mybir.ActivationFunctionType.Exp)
                    e3 = exp_t[:, :].rearrange("p (t c) -> p t c", c=C)
                    tmp1 = small_pool.tile([P, F, 4], _BF16, tag="tmp1")
                    nc.vector.tensor_add(tmp1[:, :, :], e3[:, :, 0:4],
                                         e3[:, :, 4:8])
                    tmp2 = small_pool.tile([P, F, 2], _BF16, tag="tmp2")
                    nc.vector.tensor_add(tmp2[:, :, :], tmp1[:, :, 0:2],
                                         tmp1[:, :, 2:4])
                    nc.gpsimd.tensor_add(
                        sume_b[:, ch * F:(ch + 1) * F],
                        tmp2[:, :, 0], tmp2[:, :, 1])

            # all Ln after all Exp: one ACT table-set switch instead of four
            for b in range(BPC):
                lnsc = small_pool.tile([P, COLS], _BF16, tag="lnsc")
                nc.scalar.activation(
                    lnsc[:, :], sume_all[b][:, :],
                    mybir.ActivationFunctionType.Ln,
                    accum_out=lse_acc[:, b:b + 1],
                )
                smat_sb = small_pool.tile([P, C * G], _F32, tag="smat")
                nc.vector.tensor_copy(smat_sb[:, :], psums[b][:, :])
                nc.sync.dma_start(smat_d[b], smat_sb[:, :])
            nc.sync.dma_start(lse_d[:, :], lse_acc[:, :])

    nc.compile()
    return nc


def kernel(pred, target):
    global LAST_EXEC_NS, LAST_TRACE, _nc_cache
    pred = np.asarray(pred)
    target = np.asarray(target)

    if _nc_cache is None:
        _nc_cache = _build_nc()
    nc = _nc_cache

    # pixel-major device layout: (b, ch, p, t, c)
    predv = np.asarray(pred, dtype=np.float32).reshape(B, C, P, CH, F)
    tgtf = target.reshape(B, P, COLS)
    in_maps = []
    for core in range(N_CORES):
        bs = slice(core * BPC, (core + 1) * BPC)
        pc = predv[bs].transpose(0, 3, 2, 4, 1)          # (BPC, CH, P, F, C)
        pc = np.ascontiguousarray(pc).astype(ml_dtypes.bfloat16)
        pc = pc.reshape(BPC, CH, P, F * C)
        tcore = tgtf[bs].astype(np.float32).astype(ml_dtypes.bfloat16)
        in_maps.append({"pred": pc, "target": tcore})

    res = bass_utils.run_bass_kernel_spmd(
        nc, in_maps, core_ids=list(range(N_CORES)), trace=TRACE)
    LAST_EXEC_NS = res.exec_time_ns
    LAST_TRACE = (res.instructions_and_trace[1]
                  if res.instructions_and_trace else None)

    # host combine (tiny): S[b,k,ci] = sum_g smat[k*16+g, g*8+ci]
    S = np.zeros((B, C, C), np.float64)
    total_lse = 0.0
    for core in range(N_CORES):
        smat = res.results[core]["smat"].reshape(BPC, C, G, G, C)
        S[core * BPC:(core + 1) * BPC] = np.einsum(
            "bkggc->bkc", smat.astype(np.float64))
        total_lse += res.results[core]["lse"].astype(np.float64).sum()

    n = np.zeros((B, C), np.float64)
    for b in range(B):
        n[b] = np.bincount(target[b].ravel().astype(np.int64), minlength=C)

    M = S.transpose(0, 2, 1) / n[:, None, :]             # M[b,ci,ck]
    diag = np.einsum("bcc->bc", M)
    inner = (diag[:, :, None] - M) * 0.5
    off = 1.0 - np.eye(C)
    jl = (-(np.log(0.5 + inner) * off).sum(axis=(1, 2))).mean()
    ce = (total_lse - np.einsum("bkk->", S)) / (B * N)
    return np.float32(jl + ce)


# revision 8
# speedup vs baseline: 1.7798x; 1.0471x over previous
"""J-regularized cross-entropy loss on 8 Trainium2 cores.

Math: for pred (B,C,H,W) f32, target (B,H,W) int, C=8:
  S[b,k,ci]   = sum_p pred[b,ci,p] * (target[b,p]==k)   (8x8 per batch)
  n[b,k]      = |{p: target[b,p]==k}|
  lse[b,p]    = log sum_c exp(pred[b,c,p])
  M[b,ci,ck]  = S[b,ck,ci]/n[b,ck];  jl = mean_b -sum_{ci!=ck} log(.5+.5*(diag-M))
  ce          = (sum lse - sum_b sum_k S[b,k,k]) / (B*N)
  out         = jl + ce

Device (per core, 2 batches): S via PE matmuls (one-hot weights x pred,
contracting 128 pixels/matmul, PSUM-accumulated), lse via ACT exp + DVE
add-tree + ACT ln with fused accum_out reduction. Inputs pre-converted to
bf16 on host (final scalar error ~1e-5 relative). Host finishes the tiny
(B,8,8) math in f64.

Device pred layout is pixel-major (p, t, c) so each matmul's moving
operand is a single contiguous 128-element free dim (BIR requires 1 free
dim on rhs). One-hot weights are built dg-contiguous: (p, d, k, g) so
lhsT per dg is also one contiguous 128-element slice.
"""

import numpy as np
import ml_dtypes

import concourse.bacc as bacc
import concourse.mybir as mybir
import concourse.tile as tile
from concourse import bass_utils

N_CORES = 8
B, C, H, W = 16, 8, 512, 512
N = H * W                 # 262144 pixels per batch
P = 128                   # SBUF partitions
COLS = N // P             # 2048 pixel-columns per batch
F = 1024                  # pixel-columns per chunk
CH = COLS // F            # chunks per batch
BPC = B // N_CORES        # batches per core
G = 16                    # pixel-columns per matmul group (16*8=128)
NDG = F // G              # matmuls per chunk

TRACE = False             # set True from test.py to neuron-profile
LAST_EXEC_NS = None
LAST_TRACE = None

_BF16 = mybir.dt.bfloat16
_F32 = mybir.dt.float32

_nc_cache = None


def _build_nc():
    nc = bacc.Bacc("TRN2", target_bir_lowering=False, debug=False,
                   num_devices=N_CORES)
    pred_d = nc.dram_tensor("pred", (BPC, CH, P, F * C), _BF16,
                            kind="ExternalInput")
    tgt_d = nc.dram_tensor("target", (BPC, P, COLS), _BF16,
                           kind="ExternalInput")
    smat_d = nc.dram_tensor("smat", (BPC, P, C * G), _F32,
                            kind="ExternalOutput")
    lse_d = nc.dram_tensor("lse", (P, BPC * CH), _F32,
                           kind="ExternalOutput")

    with tile.TileContext(nc) as tc:
        with (
            tc.tile_pool(name="pred", bufs=4) as pred_pool,
            tc.tile_pool(name="oh", bufs=2) as oh_pool,
            tc.tile_pool(name="exp", bufs=2) as exp_pool,
            tc.tile_pool(name="small", bufs=2) as small_pool,
            tc.tile_pool(name="acc", bufs=1) as acc_pool,
            tc.tile_pool(name="psum", bufs=2, space="PSUM") as psum_pool,
        ):
            lse_acc = acc_pool.tile([P, BPC * CH], _F32)
            sume_all = []
            for b in range(BPC):
                tgt_t = small_pool.tile([P, COLS], _BF16, tag="tgt")
                nc.sync.dma_start(tgt_t[:, :], tgt_d[b])
                psum_t = psum_pool.tile([P, C * G], _F32)
                for ch in range(CH):
                    pred_t = pred_pool.tile([P, F * C], _BF16)
                    HB = F * C // 2
                    if b == 0 and ch == 0:
                        # finer first-chunk split: start ACT/PE sooner
                        QB = HB // 2
                        for q in range(4):
                            nc.sync.dma_start(pred_t[:, q * QB:(q + 1) * QB],
                                              pred_d[b, ch, :, q * QB:(q + 1) * QB])
                    else:
                        nc.sync.dma_start(pred_t[:, :HB], pred_d[b, ch, :, :HB])
                        nc.sync.dma_start(pred_t[:, HB:], pred_d[b, ch, :, HB:])

                    # one-hot weights: oh[p, d*128 + k*16 + g] = (tgt==k)
                    oh_t = oh_pool.tile([P, NDG * C * G], _BF16)
                    oh4 = oh_t[:, :].rearrange("p (d k g) -> p d k g",
                                               k=C, g=G)
                    tgt3 = tgt_t[:, ch * F:(ch + 1) * F].rearrange(
                        "p (d g) -> p d g", g=G)
                    for k in range(C):
                        nc.vector.tensor_scalar(
                            oh4[:, :, k, :], tgt3,
                            float(k), None, mybir.AluOpType.is_equal,
                        )

                    # S: psum[k*16+g, g'*8+ci] += oh_dg^T @ pred_dg
                    for d in range(NDG):
                        nc.tensor.matmul(
                            psum_t[:, :],
                            oh_t[:, d * 128:(d + 1) * 128],
                            pred_t[:, d * 128:(d + 1) * 128],
                            start=(ch == 0 and d == 0),
                            stop=(ch == CH - 1 and d == NDG - 1),
                        )

                    # lse: exp contiguous pixel-major; class-sum via a
                    # half-split add tree whose slices stay step-1 so the
                    # bf16 DVE 2x mode applies (L1: +4 offset, L2: +2, L3: +1)
                    exp_t = exp_pool.tile([P, F * C], _BF16)
                    if b == 0 and ch == 0:
                        QB = HB // 2
                        for q in range(4):
                            nc.scalar.activation(
                                exp_t[:, q * QB:(q + 1) * QB],
                                pred_t[:, q * QB:(q + 1) * QB],
                                mybir.ActivationFunctionType.Exp)
                    else:
                        nc.scalar.activation(exp_t[:, :HB], pred_t[:, :HB],
                                             mybir.ActivationFunctionType.Exp)
                        nc.scalar.activation(exp_t[:, HB:], pred_t[:, HB:],
                                             mybir.ActivationFunctionType.Exp)
                    e3 = exp_t[:, :].rearrange("p (t c) -> p t c", c=C)
                    tmp1 = small_pool.tile([P, F, 4], _BF16, tag="tmp1")
                    nc.vector.tensor_add(tmp1[:, :, :], e3[:, :, 0:4],
                                         e3[:, :, 4:8])
                    tmp2 = small_pool.tile([P, F, 2], _BF16, tag="tmp2")
                    nc.vector.tensor_add(tmp2[:, :, :], tmp1[:, :, 0:2],
                                         tmp1[:, :, 2:4])
                    sume = acc_pool.tile([P, F], _BF16, tag=f"sume{b}{ch}")
                    sume_all.append(sume)
                    last = (b == BPC - 1 and ch == CH - 1)
                    eng = nc.vector if last else nc.gpsimd
                    eng.tensor_add(sume[:, :], tmp2[:, :, 0], tmp2[:, :, 1])

                # smat copy/DMA per batch: b0's overlaps b1's compute
                smat_sb = small_pool.tile([P, C * G], _F32, tag="smat")
                nc.vector.tensor_copy(smat_sb[:, :], psum_t[:, :])
                nc.sync.dma_start(smat_d[b], smat_sb[:, :])

            # all Ln after all Exp: one ACT table-set switch instead of four
            for i, sume in enumerate(sume_all):
                lnsc = small_pool.tile([P, F], _BF16, tag="lnsc")
                nc.scalar.activation(
                    lnsc[:, :], sume[:, :],
                    mybir.ActivationFunctionType.Ln,
                    accum_out=lse_acc[:, i:i + 1],
                )
            nc.sync.dma_start(lse_d[:, :], lse_acc[:, :])

    nc.compile()
    return nc


def kernel(pred, target):
    global LAST_EXEC_NS, LAST_TRACE, _nc_cache
    pred = np.asarray(pred)
    target = np.asarray(target)

    if _nc_cache is None:
        _nc_cache = _build_nc()
    nc = _nc_cache

    # pixel-major device layout: (b, ch, p, t, c)
    predv = np.asarray(pred, dtype=np.float32).reshape(B, C, P, CH, F)
    tgtf = target.reshape(B, P, COLS)
    in_maps = []
    for core in range(N_CORES):
        bs = slice(core * BPC, (core + 1) * BPC)
        pc = predv[bs].transpose(0, 3, 2, 4, 1)          # (BPC, CH, P, F, C)
        pc = np.ascontiguousarray(pc).astype(ml_dtypes.bfloat16)
        pc = pc.reshape(BPC, CH, P, F * C)
        tcore = tgtf[bs].astype(np.float32).astype(ml_dtypes.bfloat16)
        in_maps.append({"pred": pc, "target": tcore})

    res = bass_utils.run_bass_kernel_spmd(
        nc, in_maps, core_ids=list(range(N_CORES)), trace=TRACE)
    LAST_EXEC_NS = res.exec_time_ns
    LAST_TRACE = (res.instructions_and_trace[1]
                  if res.instructions_and_trace else None)

    # host combine (tiny): S[b,k,ci] = sum_g smat[k*16+g, g*8+ci]
    S = np.zeros((B, C, C), np.float64)
    total_lse = 0.0
    for core in range(N_CORES):
        smat = res.results[core]["smat"].reshape(BPC, C, G, G, C)
        S[core * BPC:(core + 1) * BPC] = np.einsum(
            "bkggc->bkc", smat.astype(np.float64))
        total_lse += res.results[core]["lse"].astype(np.float64).sum()

    n = np.zeros((B, C), np.float64)
    for b in range(B):
        n[b] = np.bincount(target[b].ravel().astype(np.int64), minlength=C)

    M = S.transpose(0, 2, 1) / n[:, None, :]             # M[b,ci,ck]
    diag = np.einsum("bcc->bc", M)
    inner = (diag[:, :, None] - M) * 0.5
    off = 1.0 - np.eye(C)
    jl = (-(np.log(0.5 + inner) * off).sum(axis=(1, 2))).mean()
    ce = (total_lse - np.einsum("bkk->", S)) / (B * N)
    return np.float32(jl + ce)
